# revision 6
# baseline (speedup 1.0000x reference)
"""Causal multi-head attention block on 8 TRN2 NeuronCores.

Sharding: tensor-parallel over heads (2 heads/core) for QKV projection +
attention; on-device AllToAll re-shards to sequence-parallel for the output
projection (Megatron-style). Matmuls run in float32r (full PE rate, ~1.5e-4
rel err).

Self-contained: hardcodes all shapes from the problem spec.
"""

import numpy as np
from contextlib import ExitStack

import concourse.bass as bass
import concourse.tile as tile
from concourse import bacc, mybir
from concourse.bass_utils import run_bass_kernel_spmd

F32R = mybir.dt.float32r
F32 = mybir.dt.float32
AF = mybir.ActivationFunctionType

B, T, C, H, HD = 2, 2048, 1024, 16, 64
NCORES = 8
BT = B * T            # 4096 global rows
TQ = 512              # q-chunk width
KT = 128              # k-tile height
NJ = T // TQ          # 4 q-chunks per batch
NKK = T // KT         # 16 k-tiles per batch
NCT = C // 128        # 8 contraction tiles for projections
NTC = BT // TQ        # 8 global t-chunks
TSL = BT // NCORES    # 512 rows of final output per core


def build(with_collective=True):
    nc = bacc.Bacc(None, target_bir_lowering=False)

    xt = nc.dram_tensor("xt", [C, BT], F32R, kind="ExternalInput")
    wqkv = nc.dram_tensor("wqkv", [C, 3 * 128], F32R, kind="ExternalInput")
    bqkv = nc.dram_tensor("bqkv", [128, 3], F32, kind="ExternalInput")
    wout = nc.dram_tensor("wout", [C, C], F32R, kind="ExternalInput")
    bout = nc.dram_tensor("bout", [128, C], F32, kind="ExternalInput")
    out = nc.dram_tensor("out", [TSL, C], F32, kind="ExternalOutput")

    ident_d = nc.dram_tensor("ident", [128, 128], F32R, kind="ExternalInput")
    ones_d = nc.dram_tensor("ones", [128, 64], F32R, kind="ExternalInput")
    a2a_in = nc.dram_tensor("a2a_in", [NCORES, 128, TQ], F32R)
    a2a_out = nc.dram_tensor("a2a_out", [NCORES, 128, TQ], F32R)

    with tile.TileContext(nc) as tc:
        _emit(nc, tc, xt, wqkv, bqkv, wout, bout, out, a2a_in, a2a_out,
              ident_d, ones_d, with_collective)
    nc.compile()
    return nc


def _emit(nc, tc, xt, wqkv, bqkv, wout, bout, out, a2a_in, a2a_out,
          ident_d, ones_d, with_collective):
    with ExitStack() as ctx:
        persist = ctx.enter_context(tc.tile_pool(name="persist", bufs=1))

        # persistent SBUF tensors
        qt = persist.tile([128, BT], F32R, tag="qt")    # Q^T (2 heads x 64d)
        kt_sb = persist.tile([128, BT], F32R, tag="kt")  # K^T
        va = persist.tile([128, B, NKK, 192], F32R, tag="va")  # [V_h0|ones|V_h1]
        wsb = persist.tile([128, NCT, 384], F32R, tag="wsb")
        bsb = persist.tile([128, 3], F32, tag="bsb")
        ident = persist.tile([128, 128], F32R, tag="ident")

        nc.sync.dma_start(wsb[:], wqkv[:].rearrange("(n p) c -> p n c", p=128))
        nc.sync.dma_start(bsb[:], bqkv[:])

        # identity for PE transpose + ones block of VA (from host)
        nc.sync.dma_start(ident[:], ident_d[:])
        for b0 in range(B):
            for tt0 in range(NKK):
                nc.sync.dma_start(va[:, b0, tt0, 64:128], ones_d[:])

        # ---- phase 1: QKV^T projection (+ V transpose into VA) ----
        with (
            tc.tile_pool(name="xtile", bufs=10) as xpool,
            tc.tile_pool(name="pp", bufs=6, space="PSUM") as pp,
            tc.tile_pool(name="vtile", bufs=3) as vpool,
            tc.tile_pool(name="ptr", bufs=2, space="PSUM") as ptr,
        ):
            for tc0 in range(NTC):
                b, j = tc0 // NJ, tc0 % NJ
                xts = []
                for kc in range(NCT):
                    xtile = xpool.tile([128, TQ], F32R, tag="x")
                    nc.sync.dma_start(
                        xtile[:], xt[128 * kc:128 * (kc + 1),
                                     TQ * tc0:TQ * (tc0 + 1)])
                    xts.append(xtile)
                for g in range(3):
                    ps = pp.tile([128, TQ], F32, tag="pp")
                    for kc in range(NCT):
                        nc.tensor.matmul(ps[:], wsb[:, kc, 128 * g:128 * (g + 1)],
                                         xts[kc][:],
                                         start=(kc == 0), stop=(kc == NCT - 1))
                    if g == 0:
                        nc.vector.tensor_scalar_add(
                            qt[:, TQ * tc0:TQ * (tc0 + 1)], ps[:], bsb[:, 0:1])
                    elif g == 1:
                        nc.vector.tensor_scalar_add(
                            kt_sb[:, TQ * tc0:TQ * (tc0 + 1)], ps[:], bsb[:, 1:2])
                    else:
                        vtile = vpool.tile([128, TQ], F32R, tag="v")
                        nc.vector.tensor_scalar_add(vtile[:], ps[:], bsb[:, 2:3])
                        # transpose the 4 [128,128] blocks into VA
                        for q in range(4):
                            tt = (tc0 % NJ) * 4 + q   # k-tile index in batch b
                            pst = ptr.tile([128, 128], F32R, tag="pt")
                            nc.tensor.matmul(pst[:],
                                             vtile[:, 128 * q:128 * (q + 1)],
                                             ident[:], is_transpose=True)
                            nc.vector.tensor_copy(va[:, b, tt, 0:64],
                                                  pst[:, 0:64])
                            nc.vector.tensor_copy(va[:, b, tt, 128:192],
                                                  pst[:, 64:128])

        # ---- phase 2: attention ----
        with (
            tc.tile_pool(name="ps0", bufs=1, space="PSUM") as sp0,
            tc.tile_pool(name="ps1", bufs=1, space="PSUM") as sp1,
            tc.tile_pool(name="po", bufs=4, space="PSUM") as opool,
            tc.tile_pool(name="ptp", bufs=3) as ptpool,
            tc.tile_pool(name="yt", bufs=3) as ytpool,
            tc.tile_pool(name="rt", bufs=3) as rtpool,
        ):
            spools = (sp0, sp1)
            for b in range(B):
                for j in range(NJ):
                    nkk = 4 * (j + 1)
                    po = [opool.tile([128, TQ], F32, tag="po", name=f"po{b}_{j}_{h}") for h in range(2)]
                    for g0 in range(0, nkk, 2):
                        kks = [g0, g0 + 1]
                        for h2 in range(2):
                            hp = slice(64 * h2, 64 * (h2 + 1))
                            ps_s = spools[h2].tile([128, 2 * TQ], F32,
                                                   tag=f"s{h2}")
                            for i, kk in enumerate(kks):
                                nc.tensor.matmul(
                                    ps_s[:, TQ * i:TQ * (i + 1)],
                                    kt_sb[hp, b * T + KT * kk:b * T + KT * (kk + 1)],
                                    qt[hp, b * T + TQ * j:b * T + TQ * (j + 1)],
                                    start=True, stop=True)
                            pt = ptpool.tile([128, 2 * TQ], F32R, tag=f"pt{h2}")
                            nc.scalar.activation(pt[:], ps_s[:], AF.Exp)
                            for i, kk in enumerate(kks):
                                if kk >= 4 * j:  # diagonal tile: causal mask
                                    base = TQ * j - KT * kk
                                    nc.gpsimd.affine_select(
                                        out=pt[:, TQ * i:TQ * (i + 1)],
                                        in_=pt[:, TQ * i:TQ * (i + 1)],
                                        compare_op=mybir.AluOpType.is_ge,
                                        fill=0.0, base=base,
                                        pattern=[[1, TQ]], channel_multiplier=-1)
                            for i, kk in enumerate(kks):
                                vs = slice(0, 128) if h2 == 0 else slice(64, 192)
                                nc.tensor.matmul(
                                    po[h2][:], va[:, b, kk, vs],
                                    pt[:, TQ * i:TQ * (i + 1)],
                                    start=(kk == 0), stop=(kk == nkk - 1))
                    # normalize: h0 sums in rows 64:128, h1 sums in rows 0:64
                    yt = ytpool.tile([128, TQ], F32R, tag="yt")
                    rt = rtpool.tile([128, TQ], F32, tag="rt")
                    nc.vector.reciprocal(rt[0:64, :], po[0][64:128, :])
                    nc.vector.tensor_mul(yt[0:64, :], po[0][0:64, :], rt[0:64, :])
                    nc.vector.reciprocal(rt[64:128, :], po[1][0:64, :])
                    nc.vector.tensor_mul(yt[64:128, :], po[1][64:128, :],
                                         rt[64:128, :])
                    nc.sync.dma_start(a2a_in[b * NJ + j, :, :], yt[:])

        # ---- phase 3: all-to-all (head-sharded -> t-sharded) ----
        if with_collective is True:
            nc.gpsimd.collective_compute(
                "AllToAll", mybir.AluOpType.bypass,
                replica_groups=[list(range(NCORES))],
                ins=[a2a_in[:]], outs=[a2a_out[:]])
        elif with_collective is False:
            nc.sync.dma_start(a2a_out[:], a2a_in[:])
        # else (None): timing mode — caller aliases a2a_out to a2a_in

        # ---- phase 4: output projection (rows TSL per core) ----
        with (
            tc.tile_pool(name="yts", bufs=1) as ytspool,
            tc.tile_pool(name="wo", bufs=1) as wopool,
            tc.tile_pool(name="bo", bufs=1) as bopool,
            tc.tile_pool(name="pout", bufs=4, space="PSUM") as poutp,
            tc.tile_pool(name="osb", bufs=4) as osbpool,
        ):
            yts = ytspool.tile([128, NCORES, TQ], F32R, tag="yts")
            nc.sync.dma_start(yts[:], a2a_out[:].rearrange("n p q -> p n q"))
            wosb = wopool.tile([128, NCT, C], F32R, tag="wo")
            nc.sync.dma_start(wosb[:], wout[:].rearrange("(n p) c -> p n c", p=128))
            bosb = bopool.tile([128, C], F32, tag="bo")
            nc.sync.dma_start(bosb[:], bout[:])

            for tt in range(TSL // 128):
                pos = [poutp.tile([128, TQ], F32, tag="pout", name=f"pos{tt}_{h}") for h in range(2)]
                for cc in range(NCT):
                    for n in range(2):
                        nc.tensor.matmul(
                            pos[n][:], yts[:, cc, 128 * tt:128 * (tt + 1)],
                            wosb[:, cc, TQ * n:TQ * (n + 1)],
                            start=(cc == 0), stop=(cc == NCT - 1))
                for n in range(2):
                    osb = osbpool.tile([128, TQ], F32, tag="osb")
                    nc.vector.tensor_add(osb[:], pos[n][:],
                                         bosb[:, TQ * n:TQ * (n + 1)])
                    nc.sync.dma_start(
                        out[128 * tt:128 * (tt + 1), TQ * n:TQ * (n + 1)],
                        osb[:])


def make_core_inputs(x, w_qkv, b_qkv, w_out, b_out):
    """Host-side shard/transform. Returns list of per-core input dicts."""
    x = np.asarray(x, np.float32)
    w_qkv = np.asarray(w_qkv, np.float32)
    b_qkv = np.asarray(b_qkv, np.float32)
    w_out = np.asarray(w_out, np.float32)
    b_out = np.asarray(b_out, np.float32)

    xt = np.ascontiguousarray(x.reshape(BT, C).T)
    bout_rep = np.ascontiguousarray(np.broadcast_to(b_out, (128, C)))
    in_maps = []
    for c in range(NCORES):
        s = slice(128 * c, 128 * (c + 1))
        wq = w_qkv[:, s] * 0.125
        wk = w_qkv[:, C:][:, s]
        wv = w_qkv[:, 2 * C:][:, s]
        wc = np.ascontiguousarray(np.concatenate([wq, wk, wv], axis=1))
        bc = np.ascontiguousarray(
            np.stack([b_qkv[s] * 0.125, b_qkv[C:][s], b_qkv[2 * C:][s]], axis=1))
        in_maps.append({
            "xt": xt, "wqkv": wc, "bqkv": bc,
            "wout": w_out, "bout": bout_rep,
            "ident": np.eye(128, dtype=np.float32),
            "ones": np.ones((128, 64), np.float32),
        })
    return in_maps


_NC_CACHE = {}


def kernel(x, w_qkv, b_qkv, w_out, b_out):
    if "nc" not in _NC_CACHE:
        _NC_CACHE["nc"] = build()
    nc = _NC_CACHE["nc"]
    in_maps = make_core_inputs(x, w_qkv, b_qkv, w_out, b_out)
    res = run_bass_kernel_spmd(nc, in_maps, core_ids=list(range(NCORES)))
    full = np.concatenate([res.results[c]["out"] for c in range(NCORES)], axis=0)
    return full.reshape(B, T, C)


# revision 27
# speedup vs baseline: 36.4573x; 36.4573x over previous
"""Causal multi-head attention block on 8 TRN2 NeuronCores.

Sharding: tensor-parallel over heads (2 heads/core, both batches) for the
QKV projection + attention; an on-device AllToAll re-shards to
sequence-parallel for the output projection (Megatron-style). Matmuls run in
float32r (full PE rate, ~1.5e-4 rel err). Scores use zero-padded per-head
K^T copies so they run as full K=128 matmuls (half-height K=64 row-tiled
matmuls measured 1.8x slower per-op on HW).

Self-contained: hardcodes all shapes from the problem spec.
"""

import numpy as np
from contextlib import ExitStack

import concourse.bass as bass
import concourse.tile as tile
from concourse import bacc, mybir
from concourse.bass_utils import run_bass_kernel_spmd

F32R = mybir.dt.float32r
F32 = mybir.dt.float32
AF = mybir.ActivationFunctionType

B, T, C, H, HD = 2, 2048, 1024, 16, 64
NCORES = 8
BT = B * T            # 4096 global rows
TQ = 512              # q-chunk width
KT = 128              # k-tile height
NJ = T // TQ          # 4 q-chunks per batch (= per core)
NKK = T // KT         # 16 k-tiles per batch
NCT = C // 128        # 8 contraction tiles for projections
NTC = BT // TQ        # 8 global t-chunks
TSL = BT // NCORES    # 512 rows of final output per core
SPOOL_BUFS = 2
PO_BUFS = 4
XT_SHAPE = [NCT, NTC, 128, TQ]  # tile-contiguous full x^T


def build(with_collective=True):
    nc = bacc.Bacc(None, target_bir_lowering=False)

    xt = nc.dram_tensor("xt", XT_SHAPE, F32R, kind="ExternalInput")
    wqkv = nc.dram_tensor("wqkv", [C, 3 * 128], F32R, kind="ExternalInput")
    bqkv = nc.dram_tensor("bqkv", [128, 3], F32, kind="ExternalInput")
    wout = nc.dram_tensor("wout", [C, C], F32R, kind="ExternalInput")
    bout = nc.dram_tensor("bout", [128, C], F32, kind="ExternalInput")
    out = nc.dram_tensor("out", [TSL, C], F32, kind="ExternalOutput")

    ident_d = nc.dram_tensor("ident", [128, 128], F32R, kind="ExternalInput")
    ones_d = nc.dram_tensor("ones", [128, 64], F32R, kind="ExternalInput")
    zeros_d = nc.dram_tensor("zeros", [64, TQ], F32R, kind="ExternalInput")
    a2a_in = nc.dram_tensor("a2a_in", [NCORES, 128, TQ], F32R)
    a2a_out = nc.dram_tensor("a2a_out", [NCORES, 128, TQ], F32R)

    with tile.TileContext(nc) as tc:
        _emit(nc, tc, xt, wqkv, bqkv, wout, bout, out, a2a_in, a2a_out,
              ident_d, ones_d, zeros_d, with_collective)
    nc.compile()
    return nc


def _emit(nc, tc, xt, wqkv, bqkv, wout, bout, out, a2a_in, a2a_out,
          ident_d, ones_d, zeros_d, with_collective, prefetch_wout="mid",
          trunc=None):
    with ExitStack() as ctx:
        persist = ctx.enter_context(tc.tile_pool(name="persist", bufs=1))

        # persistent SBUF tensors, indexed by batch b (the core owns the
        # same 2 heads in both batches).
        qts = [persist.tile([128, T], F32R, tag=f"qt{p}", name=f"qt{p}")
               for p in range(2)]
        # zero-padded per-head K^T (head h lives in rows 64*(h%2);
        # the other 64 rows are zero so scores run as full K=128 matmuls)
        kts = [persist.tile([128, T], F32R, tag=f"kt{h}", name=f"kt{h}")
               for h in range(4)]
        va = persist.tile([128, 2, NKK, 192], F32R, tag="va")  # [V_e|ones|V_o]
        wsb = persist.tile([128, NCT, 384], F32R, tag="wsb")
        bsb = persist.tile([128, 3], F32, tag="bsb")
        ident = persist.tile([128, 128], F32R, tag="ident")
        wosb = persist.tile([128, NCT, C], F32R, tag="wo")
        bosb = persist.tile([128, C], F32, tag="bo")

        nc.sync.dma_start(wsb[:], wqkv[:].rearrange("(n p) c -> p n c", p=128))
        nc.sync.dma_start(bsb[:], bqkv[:])
        if prefetch_wout == "early":
            nc.sync.dma_start(wosb[:], wout[:].rearrange("(n p) c -> p n c", p=128))
            nc.sync.dma_start(bosb[:], bout[:])

        # zero padding of kts + identity + VA ones (host constants)
        for h in range(4):
            dead = slice(64, 128) if h % 2 == 0 else slice(0, 64)
            for z in range(NJ):
                nc.sync.dma_start(kts[h][dead, TQ * z:TQ * (z + 1)], zeros_d[:])
        nc.sync.dma_start(ident[:], ident_d[:])
        for p0 in range(2):
            for tt0 in range(NKK):
                nc.sync.dma_start(va[:, p0, tt0, 64:128], ones_d[:])

        # ---- phase 1: QKV^T projection (+ V transpose into VA) ----
        with (
            tc.tile_pool(name="xtile", bufs=24) as xpool,
            tc.tile_pool(name="pp", bufs=6, space="PSUM") as pp,
            tc.tile_pool(name="vtile", bufs=3) as vpool,
            tc.tile_pool(name="ptr", bufs=2, space="PSUM") as ptr,
        ):
            for tc0 in range(NTC):
                b, jloc = tc0 // NJ, tc0 % NJ
                xts = []
                for kc in range(NCT):
                    xtile = xpool.tile([128, TQ], F32R, tag="x",
                                       name=f"x{tc0}_{kc}")
                    nc.sync.dma_start(xtile[:], xt[kc, tc0])
                    xts.append(xtile)
                for g in range(3):
                    gcol = 128 * g
                    ps = pp.tile([128, TQ], F32, tag="pp", name=f"pp{tc0}_{g}")
                    for kc in range(NCT):
                        nc.tensor.matmul(ps[:], wsb[:, kc, gcol:gcol + 128],
                                         xts[kc][:],
                                         start=(kc == 0), stop=(kc == NCT - 1))
                    chunk = slice(TQ * jloc, TQ * (jloc + 1))
                    if g == 0:        # Q^T of batch b
                        nc.vector.tensor_scalar_add(qts[b][:, chunk], ps[:],
                                                    bsb[:, 0:1])
                    elif g == 1:      # K^T of batch b, split per head
                        nc.vector.tensor_scalar_add(
                            kts[2 * b][0:64, chunk], ps[0:64, :],
                            bsb[0:64, 1:2])
                        nc.vector.tensor_scalar_add(
                            kts[2 * b + 1][64:128, chunk], ps[64:128, :],
                            bsb[64:128, 1:2])
                    else:             # V of batch b -> transpose into VA
                        vtile = vpool.tile([128, TQ], F32R, tag="v",
                                           name=f"v{tc0}")
                        nc.vector.tensor_scalar_add(vtile[:], ps[:],
                                                    bsb[:, 2:3])
                        for q in range(4):
                            tt = jloc * 4 + q   # k-tile index in batch b
                            pst = ptr.tile([128, 128], F32R, tag="pt",
                                           name=f"pt{tc0}_{q}")
                            nc.tensor.matmul(pst[:],
                                             vtile[:, 128 * q:128 * (q + 1)],
                                             ident[:], is_transpose=True)
                            nc.vector.tensor_copy(va[:, b, tt, 0:64],
                                                  pst[:, 0:64])
                            nc.vector.tensor_copy(va[:, b, tt, 128:192],
                                                  pst[:, 64:128])

        if trunc == "proj":
            with tc.tile_pool(name="dumo", bufs=1) as dpool:
                d = dpool.tile([128, TQ], F32, tag="d")
                nc.vector.tensor_copy(d[:], qts[0][0:128, 0:TQ].bitcast(F32))
                nc.sync.dma_start(out[0:128, 0:TQ], d[:])
            return

        # ---- phase 2: attention (p = batch index) ----
        with (
            tc.tile_pool(name="psc", bufs=SPOOL_BUFS, space="PSUM") as spool,
            tc.tile_pool(name="po", bufs=PO_BUFS, space="PSUM") as opool,
            tc.tile_pool(name="ptp", bufs=6) as ptpool,
            tc.tile_pool(name="yt", bufs=3) as ytpool,
            tc.tile_pool(name="rt", bufs=3) as rtpool,
        ):
            for p in range(2):
                if p == 1 and prefetch_wout == "mid":
                    nc.sync.dma_start(wosb[:],
                                      wout[:].rearrange("(n p) c -> p n c", p=128))
                    nc.sync.dma_start(bosb[:], bout[:])
                for j in range(NJ):
                    nkk = 4 * (j + 1)
                    po = [opool.tile([128, TQ], F32, tag="po",
                                     name=f"po{p}_{j}_{h}") for h in range(2)]
                    for kk in range(nkk):
                        ps_s = spool.tile([128, 2 * TQ], F32, tag="s",
                                          name=f"s{p}_{j}_{kk}")
                        for h2 in range(2):
                            nc.tensor.matmul(
                                ps_s[:, TQ * h2:TQ * (h2 + 1)],
                                kts[2 * p + h2][:, KT * kk:KT * (kk + 1)],
                                qts[p][:, TQ * j:TQ * (j + 1)],
                                start=True, stop=True)
                        pt = ptpool.tile([128, 2 * TQ], F32R, tag="pt",
                                         name=f"p{p}_{j}_{kk}")
                        nc.scalar.activation(pt[:], ps_s[:], AF.Exp)
                        o = max(kk - 4 * j, 0)  # suffix offset (diag tiles)
                        if kk >= 4 * j:
                            for h2 in range(2):
                                lo = TQ * h2 + KT * o
                                # aligned triangle: keep qf' >= r
                                nc.gpsimd.affine_select(
                                    out=pt[:, lo:TQ * (h2 + 1)],
                                    in_=pt[:, lo:TQ * (h2 + 1)],
                                    compare_op=mybir.AluOpType.is_ge,
                                    fill=0.0, base=0,
                                    pattern=[[1, TQ - KT * o]],
                                    channel_multiplier=-1)
                        for h2 in range(2):
                            vs = slice(0, 128) if h2 == 0 else slice(64, 192)
                            nc.tensor.matmul(
                                po[h2][:, KT * o:TQ],
                                va[:, p, kk, vs],
                                pt[:, TQ * h2 + KT * o:TQ * (h2 + 1)],
                                start=(kk == 0), stop=(kk == nkk - 1))
                    # normalize: h0 sums in rows 64:128, h1 sums in rows 0:64
                    yt = ytpool.tile([128, TQ], F32R, tag="yt", name=f"y{p}_{j}")
                    rt = rtpool.tile([128, TQ], F32, tag="rt", name=f"r{p}_{j}")
                    nc.vector.reciprocal(rt[0:64, :], po[0][64:128, :])
                    nc.vector.tensor_mul(yt[0:64, :], po[0][0:64, :], rt[0:64, :])
                    nc.vector.reciprocal(rt[64:128, :], po[1][0:64, :])
                    nc.vector.tensor_mul(yt[64:128, :], po[1][64:128, :],
                                         rt[64:128, :])
                    nc.sync.dma_start(a2a_in[p * NJ + j, :, :], yt[:])

        if trunc == "attn":
            with tc.tile_pool(name="dumo2", bufs=1) as dpool2:
                d2 = dpool2.tile([128, TQ], F32, tag="d2")
                nc.vector.tensor_copy(d2[:], qts[0][0:128, 0:TQ].bitcast(F32))
                nc.sync.dma_start(out[0:128, 0:TQ], d2[:])
            return

        # ---- phase 3: all-to-all (head-sharded -> t-sharded) ----
        if with_collective is True:
            nc.gpsimd.collective_compute(
                "AllToAll", mybir.AluOpType.bypass,
                replica_groups=[list(range(NCORES))],
                ins=[a2a_in[:]], outs=[a2a_out[:]])
        elif with_collective is False:
            nc.sync.dma_start(a2a_out[:], a2a_in[:])
        # else (None): timing mode — caller aliases a2a_out to a2a_in

        # ---- phase 4: output projection (rows TSL per core) ----
        with (
            tc.tile_pool(name="yts", bufs=1) as ytspool,
            tc.tile_pool(name="pout", bufs=4, space="PSUM") as poutp,
            tc.tile_pool(name="osb", bufs=4) as osbpool,
        ):
            yts = ytspool.tile([128, NCT, TQ], F32R, tag="yts")
            for cc in range(NCT):
                nc.sync.dma_start(yts[:, cc, :], a2a_out[cc, :, :])

            for tt in range(TSL // 128):
                pos = [poutp.tile([128, TQ], F32, tag="pout",
                                  name=f"pos{tt}_{h}") for h in range(2)]
                for cc in range(NCT):
                    for n in range(2):
                        nc.tensor.matmul(
                            pos[n][:], yts[:, cc, 128 * tt:128 * (tt + 1)],
                            wosb[:, cc, TQ * n:TQ * (n + 1)],
                            start=(cc == 0), stop=(cc == NCT - 1))
                for n in range(2):
                    osb = osbpool.tile([128, TQ], F32, tag="osb")
                    nc.vector.tensor_add(osb[:], pos[n][:],
                                         bosb[:, TQ * n:TQ * (n + 1)])
                    nc.sync.dma_start(
                        out[128 * tt:128 * (tt + 1), TQ * n:TQ * (n + 1)],
                        osb[:])


def make_core_inputs(x, w_qkv, b_qkv, w_out, b_out):
    """Host-side shard/transform. Returns list of per-core input dicts."""
    x = np.asarray(x, np.float32)
    w_qkv = np.asarray(w_qkv, np.float32)
    b_qkv = np.asarray(b_qkv, np.float32)
    w_out = np.asarray(w_out, np.float32)
    b_out = np.asarray(b_out, np.float32)

    bout_rep = np.ascontiguousarray(np.broadcast_to(b_out, (128, C)))
    # tile-contiguous x^T: xt[kc, tc0, p, q] = x_flat[TQ*tc0+q, 128*kc+p]
    xt = np.ascontiguousarray(
        x.reshape(NTC, TQ, NCT, 128).transpose(2, 0, 3, 1))
    in_maps = []
    for c in range(NCORES):
        s = slice(128 * c, 128 * (c + 1))
        wq = w_qkv[:, :C][:, s] * 0.125
        wk = w_qkv[:, C:2 * C][:, s]
        wv = w_qkv[:, 2 * C:][:, s]
        wc = np.ascontiguousarray(np.concatenate([wq, wk, wv], axis=1))
        bc3 = np.ascontiguousarray(
            np.stack([b_qkv[:C][s] * 0.125, b_qkv[C:2 * C][s],
                      b_qkv[2 * C:][s]], axis=1))
        in_maps.append({
            "xt": xt, "wqkv": wc, "bqkv": bc3,
            "wout": w_out, "bout": bout_rep,
            "ident": np.eye(128, dtype=np.float32),
            "ones": np.ones((128, 64), np.float32),
            "zeros": np.zeros((64, TQ), np.float32),
        })
    return in_maps


_NC_CACHE = {}


def _make_cached_runner(nc):
    """Jit the SPMD executable once; subsequent calls only re-upload inputs."""
    import jax
    from jax.sharding import Mesh, PartitionSpec
    from jax.experimental.shard_map import shard_map
    from concourse.bass2jax import _bass_exec_p, install_neuronx_cc_hook

    install_neuronx_cc_hook()
    in_names, out_names, out_avals = [], [], []
    for alloc in nc.m.functions[0].allocations:
        if not isinstance(alloc, mybir.MemoryLocationSet):
            continue
        name = alloc.memorylocations[0].name
        if alloc.kind == "ExternalInput":
            in_names.append(name)
        elif alloc.kind == "ExternalOutput":
            out_names.append(name)
            out_avals.append(jax.core.ShapedArray(
                tuple(alloc.tensor_shape), mybir.dt.np(alloc.dtype)))
    n_params = len(in_names)
    all_in = list(in_names) + list(out_names)

    def _body(*args):
        outs = _bass_exec_p.bind(
            *args, out_avals=tuple(out_avals), in_names=tuple(all_in),
            out_names=tuple(out_names), lowering_input_output_aliases=(),
            sim_require_finite=True, sim_require_nnan=True, nc=nc)
        return tuple(outs)

    devices = jax.devices()[:NCORES]
    mesh = Mesh(np.asarray(devices), ("core",))
    spec = PartitionSpec("core")
    sharded = jax.jit(
        shard_map(_body, mesh=mesh,
                  in_specs=(spec,) * (n_params + len(out_names)),
                  out_specs=(spec,) * len(out_names), check_rep=False),
        keep_unused=True)
    zeros = [np.zeros((NCORES * a.shape[0], *a.shape[1:]), a.dtype)
             for a in out_avals]

    def run(in_maps):
        concat = [np.concatenate([np.asarray(m[nm]) for m in in_maps], axis=0)
                  for nm in in_names]
        outs = sharded(*concat, *zeros)
        return {nm: np.asarray(outs[i]) for i, nm in enumerate(out_names)}

    return run


def kernel(x, w_qkv, b_qkv, w_out, b_out):
    in_maps = make_core_inputs(x, w_qkv, b_qkv, w_out, b_out)
    if "nc" not in _NC_CACHE:
        _NC_CACHE["nc"] = build()
    nc = _NC_CACHE["nc"]
    try:
        if "run" not in _NC_CACHE:
            _NC_CACHE["run"] = _make_cached_runner(nc)
        outs = _NC_CACHE["run"](in_maps)
        full = outs["out"].reshape(NCORES * TSL, C)
    except Exception:
        res = run_bass_kernel_spmd(nc, in_maps, core_ids=list(range(NCORES)))
        full = np.concatenate([res.results[c]["out"] for c in range(NCORES)],
                              axis=0)
    return full.reshape(B, T, C)


# revision 29
# speedup vs baseline: 39.3778x; 1.0801x over previous
"""Causal multi-head attention block on 8 TRN2 NeuronCores.

Sharding: tensor-parallel over heads (2 heads/core, both batches) for the
QKV projection + attention; an on-device AllToAll re-shards to
sequence-parallel for the output projection (Megatron-style). Matmuls run in
float32r (full PE rate, ~1.5e-4 rel err). Scores use zero-padded per-head
K^T copies so they run as full K=128 matmuls (half-height K=64 row-tiled
matmuls measured 1.8x slower per-op on HW).

Self-contained: hardcodes all shapes from the problem spec.
"""

import numpy as np
from contextlib import ExitStack

import concourse.bass as bass
import concourse.tile as tile
from concourse import bacc, mybir
from concourse.bass_utils import run_bass_kernel_spmd

F32R = mybir.dt.float32r
F32 = mybir.dt.float32
AF = mybir.ActivationFunctionType

B, T, C, H, HD = 2, 2048, 1024, 16, 64
NCORES = 8
BT = B * T            # 4096 global rows
TQ = 512              # q-chunk width
KT = 128              # k-tile height
NJ = T // TQ          # 4 q-chunks per batch (= per core)
NKK = T // KT         # 16 k-tiles per batch
NCT = C // 128        # 8 contraction tiles for projections
NTC = BT // TQ        # 8 global t-chunks
TSL = BT // NCORES    # 512 rows of final output per core
SPOOL_BUFS = 3
PO_BUFS = 2
QUICK_EVICT = True
XT_SHAPE = [NCT, NTC, 128, TQ]  # tile-contiguous full x^T


def build(with_collective=True):
    nc = bacc.Bacc(None, target_bir_lowering=False)

    xt = nc.dram_tensor("xt", XT_SHAPE, F32R, kind="ExternalInput")
    wqkv = nc.dram_tensor("wqkv", [C, 3 * 128], F32R, kind="ExternalInput")
    bqkv = nc.dram_tensor("bqkv", [128, 3], F32, kind="ExternalInput")
    wout = nc.dram_tensor("wout", [C, C], F32R, kind="ExternalInput")
    bout = nc.dram_tensor("bout", [128, C], F32, kind="ExternalInput")
    out = nc.dram_tensor("out", [TSL, C], F32, kind="ExternalOutput")

    ident_d = nc.dram_tensor("ident", [128, 128], F32R, kind="ExternalInput")
    ones_d = nc.dram_tensor("ones", [128, 64], F32R, kind="ExternalInput")
    zeros_d = nc.dram_tensor("zeros", [64, TQ], F32R, kind="ExternalInput")
    a2a_in = nc.dram_tensor("a2a_in", [NCORES, 128, TQ], F32R)
    a2a_out = nc.dram_tensor("a2a_out", [NCORES, 128, TQ], F32R)

    with tile.TileContext(nc) as tc:
        _emit(nc, tc, xt, wqkv, bqkv, wout, bout, out, a2a_in, a2a_out,
              ident_d, ones_d, zeros_d, with_collective)
    nc.compile()
    return nc


def _emit(nc, tc, xt, wqkv, bqkv, wout, bout, out, a2a_in, a2a_out,
          ident_d, ones_d, zeros_d, with_collective, prefetch_wout="mid",
          trunc=None):
    with ExitStack() as ctx:
        persist = ctx.enter_context(tc.tile_pool(name="persist", bufs=1))

        # persistent SBUF tensors, indexed by batch b (the core owns the
        # same 2 heads in both batches).
        qts = [persist.tile([128, T], F32R, tag=f"qt{p}", name=f"qt{p}")
               for p in range(2)]
        # zero-padded per-head K^T (head h lives in rows 64*(h%2);
        # the other 64 rows are zero so scores run as full K=128 matmuls)
        kts = [persist.tile([128, T], F32R, tag=f"kt{h}", name=f"kt{h}")
               for h in range(4)]
        va = persist.tile([128, 2, NKK, 192], F32R, tag="va")  # [V_e|ones|V_o]
        wsb = persist.tile([128, NCT, 384], F32R, tag="wsb")
        bsb = persist.tile([128, 3], F32, tag="bsb")
        ident = persist.tile([128, 128], F32R, tag="ident")
        wosb = persist.tile([128, NCT, C], F32R, tag="wo")
        bosb = persist.tile([128, C], F32, tag="bo")

        nc.sync.dma_start(wsb[:], wqkv[:].rearrange("(n p) c -> p n c", p=128))
        nc.sync.dma_start(bsb[:], bqkv[:])
        if prefetch_wout == "early":
            nc.sync.dma_start(wosb[:], wout[:].rearrange("(n p) c -> p n c", p=128))
            nc.sync.dma_start(bosb[:], bout[:])

        # zero padding of kts + identity + VA ones (host constants)
        for h in range(4):
            dead = slice(64, 128) if h % 2 == 0 else slice(0, 64)
            for z in range(NJ):
                nc.sync.dma_start(kts[h][dead, TQ * z:TQ * (z + 1)], zeros_d[:])
        nc.sync.dma_start(ident[:], ident_d[:])
        for p0 in range(2):
            for tt0 in range(NKK):
                nc.sync.dma_start(va[:, p0, tt0, 64:128], ones_d[:])

        # ---- phase 1: QKV^T projection (+ V transpose into VA) ----
        with (
            tc.tile_pool(name="xtile", bufs=24) as xpool,
            tc.tile_pool(name="pp", bufs=6, space="PSUM") as pp,
            tc.tile_pool(name="vtile", bufs=3) as vpool,
            tc.tile_pool(name="ptr", bufs=2, space="PSUM") as ptr,
        ):
            for tc0 in range(NTC):
                b, jloc = tc0 // NJ, tc0 % NJ
                xts = []
                for kc in range(NCT):
                    xtile = xpool.tile([128, TQ], F32R, tag="x",
                                       name=f"x{tc0}_{kc}")
                    nc.sync.dma_start(xtile[:], xt[kc, tc0])
                    xts.append(xtile)
                for g in range(3):
                    gcol = 128 * g
                    ps = pp.tile([128, TQ], F32, tag="pp", name=f"pp{tc0}_{g}")
                    for kc in range(NCT):
                        nc.tensor.matmul(ps[:], wsb[:, kc, gcol:gcol + 128],
                                         xts[kc][:],
                                         start=(kc == 0), stop=(kc == NCT - 1))
                    chunk = slice(TQ * jloc, TQ * (jloc + 1))
                    if g == 0:        # Q^T of batch b
                        nc.vector.tensor_scalar_add(qts[b][:, chunk], ps[:],
                                                    bsb[:, 0:1])
                    elif g == 1:      # K^T of batch b, split per head
                        nc.vector.tensor_scalar_add(
                            kts[2 * b][0:64, chunk], ps[0:64, :],
                            bsb[0:64, 1:2])
                        nc.vector.tensor_scalar_add(
                            kts[2 * b + 1][64:128, chunk], ps[64:128, :],
                            bsb[64:128, 1:2])
                    else:             # V of batch b -> transpose into VA
                        vtile = vpool.tile([128, TQ], F32R, tag="v",
                                           name=f"v{tc0}")
                        nc.vector.tensor_scalar_add(vtile[:], ps[:],
                                                    bsb[:, 2:3])
                        for q in range(4):
                            tt = jloc * 4 + q   # k-tile index in batch b
                            pst = ptr.tile([128, 128], F32R, tag="pt",
                                           name=f"pt{tc0}_{q}")
                            nc.tensor.matmul(pst[:],
                                             vtile[:, 128 * q:128 * (q + 1)],
                                             ident[:], is_transpose=True)
                            nc.vector.tensor_copy(va[:, b, tt, 0:64],
                                                  pst[:, 0:64])
                            nc.vector.tensor_copy(va[:, b, tt, 128:192],
                                                  pst[:, 64:128])

        if trunc == "proj":
            with tc.tile_pool(name="dumo", bufs=1) as dpool:
                d = dpool.tile([128, TQ], F32, tag="d")
                nc.vector.tensor_copy(d[:], qts[0][0:128, 0:TQ].bitcast(F32))
                nc.sync.dma_start(out[0:128, 0:TQ], d[:])
            return

        # ---- phase 2: attention (p = batch index) ----
        with (
            tc.tile_pool(name="psc", bufs=SPOOL_BUFS, space="PSUM") as spool,
            tc.tile_pool(name="po", bufs=PO_BUFS, space="PSUM") as opool,
            tc.tile_pool(name="ptp", bufs=6) as ptpool,
            tc.tile_pool(name="yt", bufs=3) as ytpool,
            tc.tile_pool(name="rt", bufs=3) as rtpool,
            tc.tile_pool(name="oe", bufs=4) as oepool,
        ):
            for p in range(2):
                if p == 1 and prefetch_wout == "mid":
                    nc.sync.dma_start(wosb[:],
                                      wout[:].rearrange("(n p) c -> p n c", p=128))
                    nc.sync.dma_start(bosb[:], bout[:])
                for j in range(NJ):
                    nkk = 4 * (j + 1)
                    po = [opool.tile([128, TQ], F32, tag="po",
                                     name=f"po{p}_{j}_{h}") for h in range(2)]
                    for kk in range(nkk):
                        ps_s = spool.tile([128, 2 * TQ], F32, tag="s",
                                          name=f"s{p}_{j}_{kk}")
                        for h2 in range(2):
                            nc.tensor.matmul(
                                ps_s[:, TQ * h2:TQ * (h2 + 1)],
                                kts[2 * p + h2][:, KT * kk:KT * (kk + 1)],
                                qts[p][:, TQ * j:TQ * (j + 1)],
                                start=True, stop=True)
                        pt = ptpool.tile([128, 2 * TQ], F32R, tag="pt",
                                         name=f"p{p}_{j}_{kk}")
                        nc.scalar.activation(pt[:], ps_s[:], AF.Exp)
                        o = max(kk - 4 * j, 0)  # suffix offset (diag tiles)
                        if kk >= 4 * j:
                            for h2 in range(2):
                                lo = TQ * h2 + KT * o
                                # aligned triangle: keep qf' >= r
                                nc.gpsimd.affine_select(
                                    out=pt[:, lo:TQ * (h2 + 1)],
                                    in_=pt[:, lo:TQ * (h2 + 1)],
                                    compare_op=mybir.AluOpType.is_ge,
                                    fill=0.0, base=0,
                                    pattern=[[1, TQ - KT * o]],
                                    channel_multiplier=-1)
                        for h2 in range(2):
                            vs = slice(0, 128) if h2 == 0 else slice(64, 192)
                            nc.tensor.matmul(
                                po[h2][:, KT * o:TQ],
                                va[:, p, kk, vs],
                                pt[:, TQ * h2 + KT * o:TQ * (h2 + 1)],
                                start=(kk == 0), stop=(kk == nkk - 1))
                    # normalize: h0 sums in rows 64:128, h1 sums in rows 0:64
                    yt = ytpool.tile([128, TQ], F32R, tag="yt", name=f"y{p}_{j}")
                    rt = rtpool.tile([128, TQ], F32, tag="rt", name=f"r{p}_{j}")
                    if QUICK_EVICT:
                        # copy psum->sbuf fast so the accumulator banks free
                        # for the next q-chunk before the recip/mul run
                        oes = [oepool.tile([128, TQ], F32, tag="oe",
                                           name=f"oe{p}_{j}_{h}")
                               for h in range(2)]
                        nc.vector.tensor_copy(oes[0][:], po[0][:])
                        nc.vector.tensor_copy(oes[1][:], po[1][:])
                        src0, src1 = oes[0], oes[1]
                    else:
                        src0, src1 = po[0], po[1]
                    nc.vector.reciprocal(rt[0:64, :], src0[64:128, :])
                    nc.vector.tensor_mul(yt[0:64, :], src0[0:64, :], rt[0:64, :])
                    nc.vector.reciprocal(rt[64:128, :], src1[0:64, :])
                    nc.vector.tensor_mul(yt[64:128, :], src1[64:128, :],
                                         rt[64:128, :])
                    nc.sync.dma_start(a2a_in[p * NJ + j, :, :], yt[:])

        if trunc == "attn":
            with tc.tile_pool(name="dumo2", bufs=1) as dpool2:
                d2 = dpool2.tile([128, TQ], F32, tag="d2")
                nc.vector.tensor_copy(d2[:], qts[0][0:128, 0:TQ].bitcast(F32))
                nc.sync.dma_start(out[0:128, 0:TQ], d2[:])
            return

        # ---- phase 3: all-to-all (head-sharded -> t-sharded) ----
        if with_collective is True:
            nc.gpsimd.collective_compute(
                "AllToAll", mybir.AluOpType.bypass,
                replica_groups=[list(range(NCORES))],
                ins=[a2a_in[:]], outs=[a2a_out[:]])
        elif with_collective is False:
            nc.sync.dma_start(a2a_out[:], a2a_in[:])
        # else (None): timing mode — caller aliases a2a_out to a2a_in

        # ---- phase 4: output projection (rows TSL per core) ----
        with (
            tc.tile_pool(name="yts", bufs=1) as ytspool,
            tc.tile_pool(name="pout", bufs=4, space="PSUM") as poutp,
            tc.tile_pool(name="osb", bufs=4) as osbpool,
        ):
            yts = ytspool.tile([128, NCT, TQ], F32R, tag="yts")
            for cc in range(NCT):
                nc.sync.dma_start(yts[:, cc, :], a2a_out[cc, :, :])

            for tt in range(TSL // 128):
                pos = [poutp.tile([128, TQ], F32, tag="pout",
                                  name=f"pos{tt}_{h}") for h in range(2)]
                for cc in range(NCT):
                    for n in range(2):
                        nc.tensor.matmul(
                            pos[n][:], yts[:, cc, 128 * tt:128 * (tt + 1)],
                            wosb[:, cc, TQ * n:TQ * (n + 1)],
                            start=(cc == 0), stop=(cc == NCT - 1))
                for n in range(2):
                    osb = osbpool.tile([128, TQ], F32, tag="osb")
                    nc.vector.tensor_add(osb[:], pos[n][:],
                                         bosb[:, TQ * n:TQ * (n + 1)])
                    nc.sync.dma_start(
                        out[128 * tt:128 * (tt + 1), TQ * n:TQ * (n + 1)],
                        osb[:])


def make_core_inputs(x, w_qkv, b_qkv, w_out, b_out):
    """Host-side shard/transform. Returns list of per-core input dicts."""
    x = np.asarray(x, np.float32)
    w_qkv = np.asarray(w_qkv, np.float32)
    b_qkv = np.asarray(b_qkv, np.float32)
    w_out = np.asarray(w_out, np.float32)
    b_out = np.asarray(b_out, np.float32)

    bout_rep = np.ascontiguousarray(np.broadcast_to(b_out, (128, C)))
    # tile-contiguous x^T: xt[kc, tc0, p, q] = x_flat[TQ*tc0+q, 128*kc+p]
    xt = np.ascontiguousarray(
        x.reshape(NTC, TQ, NCT, 128).transpose(2, 0, 3, 1))
    in_maps = []
    for c in range(NCORES):
        s = slice(128 * c, 128 * (c + 1))
        wq = w_qkv[:, :C][:, s] * 0.125
        wk = w_qkv[:, C:2 * C][:, s]
        wv = w_qkv[:, 2 * C:][:, s]
        wc = np.ascontiguousarray(np.concatenate([wq, wk, wv], axis=1))
        bc3 = np.ascontiguousarray(
            np.stack([b_qkv[:C][s] * 0.125, b_qkv[C:2 * C][s],
                      b_qkv[2 * C:][s]], axis=1))
        in_maps.append({
            "xt": xt, "wqkv": wc, "bqkv": bc3,
            "wout": w_out, "bout": bout_rep,
            "ident": np.eye(128, dtype=np.float32),
            "ones": np.ones((128, 64), np.float32),
            "zeros": np.zeros((64, TQ), np.float32),
        })
    return in_maps


_NC_CACHE = {}


def _make_cached_runner(nc):
    """Jit the SPMD executable once; subsequent calls only re-upload inputs."""
    import jax
    from jax.sharding import Mesh, PartitionSpec
    from jax.experimental.shard_map import shard_map
    from concourse.bass2jax import _bass_exec_p, install_neuronx_cc_hook

    install_neuronx_cc_hook()
    in_names, out_names, out_avals = [], [], []
    for alloc in nc.m.functions[0].allocations:
        if not isinstance(alloc, mybir.MemoryLocationSet):
            continue
        name = alloc.memorylocations[0].name
        if alloc.kind == "ExternalInput":
            in_names.append(name)
        elif alloc.kind == "ExternalOutput":
            out_names.append(name)
            out_avals.append(jax.core.ShapedArray(
                tuple(alloc.tensor_shape), mybir.dt.np(alloc.dtype)))
    n_params = len(in_names)
    all_in = list(in_names) + list(out_names)

    def _body(*args):
        outs = _bass_exec_p.bind(
            *args, out_avals=tuple(out_avals), in_names=tuple(all_in),
            out_names=tuple(out_names), lowering_input_output_aliases=(),
            sim_require_finite=True, sim_require_nnan=True, nc=nc)
        return tuple(outs)

    devices = jax.devices()[:NCORES]
    mesh = Mesh(np.asarray(devices), ("core",))
    spec = PartitionSpec("core")
    sharded = jax.jit(
        shard_map(_body, mesh=mesh,
                  in_specs=(spec,) * (n_params + len(out_names)),
                  out_specs=(spec,) * len(out_names), check_rep=False),
        keep_unused=True)
    zeros = [np.zeros((NCORES * a.shape[0], *a.shape[1:]), a.dtype)
             for a in out_avals]

    def run(in_maps):
        concat = [np.concatenate([np.asarray(m[nm]) for m in in_maps], axis=0)
                  for nm in in_names]
        outs = sharded(*concat, *zeros)
        return {nm: np.asarray(outs[i]) for i, nm in enumerate(out_names)}

    return run


def kernel(x, w_qkv, b_qkv, w_out, b_out):
    in_maps = make_core_inputs(x, w_qkv, b_qkv, w_out, b_out)
    if "nc" not in _NC_CACHE:
        _NC_CACHE["nc"] = build()
    nc = _NC_CACHE["nc"]
    try:
        if "run" not in _NC_CACHE:
            _NC_CACHE["run"] = _make_cached_runner(nc)
        outs = _NC_CACHE["run"](in_maps)
        full = outs["out"].reshape(NCORES * TSL, C)
    except Exception:
        res = run_bass_kernel_spmd(nc, in_maps, core_ids=list(range(NCORES)))
        full = np.concatenate([res.results[c]["out"] for c in range(NCORES)],
                              axis=0)
    return full.reshape(B, T, C)
